# revision 5
# baseline (speedup 1.0000x reference)
"""ForgetMult (h_t = f_t*h_{t-1} + (1-f_t)*z_t) on 8 TRN2 NeuronCores.

Full inputs f, z: [T=1024, B=32, H=1024] f32. Output h: [T, B, H].

Sharding: batch dim across the 8 cores (4 batches/core), no communication.
Per core the problem is [T=1024, N=4096] with an independent linear
recurrence along T for each of the N columns.

Per-core dataflow (per n-group of W=512 columns):
  - one DMA per tensor brings the whole [T, W] panel in as a
    [128, T/128, W] t-block-interleaved SBUF tile (2 KiB rows)
  - DVE: bneg = (f - 1) * z -> bf16 (one scalar_tensor_tensor op)
  - GPSIMD (otherwise idle): split f into bf16 limbs f_hi = bf16(f),
    f_lo = bf16(f - f_hi)  (2-limb bf16 keeps f at ~8e-6 relative)
  - PE: "transpose" 128x128 blocks via REGULAR matmuls against a bf16
    identity (block.T @ I). Regular bf16 matmuls run warm at ~56 ns vs
    ~286 ns for dtype-insensitive transpose-mode. f's two limbs
    accumulate into one fp32 PSUM tile, reconstructing full precision.
  - DVE: tensor_tensor_scan along t: state = f*state - bneg (fp32 state),
    data0 = f_tr straight from PSUM, data1 = bneg_tr copied to SBUF by ACT
  - PE: h (fp32) transposed back via transpose-mode; ACT copies PSUM->SBUF
  - one DMA per group writes the h panel out

Only bneg is bf16 (additive input, ~1e-3 on h); f, the scan state, and the
whole h output path stay effectively fp32.
"""

from contextlib import ExitStack

import numpy as np

T, B, H = 1024, 32, 1024
NCORES = 8
BPC = B // NCORES  # 4 batches per core
N = BPC * H  # 4096 recurrence columns per core
P = 128

W_FULL = 512  # panel width (columns per n-group)


def build_forget_mult(tc, h_d, f_d, z_d, i_d, ib_d, ctx, t_sz, n_sz, w_sz):
    """Emit the per-core Tile program. f_d/z_d/h_d are DRAM APs [t_sz, n_sz]."""
    import concourse.bass as bass
    from concourse import mybir

    nc = tc.nc
    fp32 = mybir.dt.float32
    bf16 = mybir.dt.bfloat16
    su = mybir.AluOpType.subtract
    mu = mybir.AluOpType.mult

    tb = t_sz // P  # t-blocks (8)
    ng = n_sz // w_sz  # n-groups (8)
    nb = w_sz // P  # n-blocks per group (4)
    n_halves = 2  # scan chunks along T
    thb = tb // n_halves  # t-blocks per half (4)
    scan_len = thb * P  # 512
    assert t_sz % P == 0 and n_sz % w_sz == 0 and w_sz % P == 0 and tb % 2 == 0

    const_pool = ctx.enter_context(tc.tile_pool(name="const", bufs=1))
    ident = const_pool.tile([P, P], fp32)
    nc.sync.dma_start(ident[:], i_d[:])
    identb = const_pool.tile([P, P], bf16)
    nc.sync.dma_start(identb[:], ib_d[:])

    f_pool = ctx.enter_context(tc.tile_pool(name="fpanel", bufs=2))
    z_pool = ctx.enter_context(tc.tile_pool(name="zpanel", bufs=2))
    b_pool = ctx.enter_context(tc.tile_pool(name="bpanel", bufs=2))
    fh_pool = ctx.enter_context(tc.tile_pool(name="fhpanel", bufs=2))
    fl_pool = ctx.enter_context(tc.tile_pool(name="flpanel", bufs=2))
    h_pool = ctx.enter_context(tc.tile_pool(name="hpanel", bufs=2))
    btr_s_pool = ctx.enter_context(tc.tile_pool(name="btrs", bufs=2))
    htr_pool = ctx.enter_context(tc.tile_pool(name="htr", bufs=2 * nb))
    ftr_p_pool = ctx.enter_context(tc.tile_pool(name="ftrp", bufs=2, space="PSUM"))
    btr_p_pool = ctx.enter_context(tc.tile_pool(name="btrp", bufs=2, space="PSUM"))
    hbk_p_pool = ctx.enter_context(tc.tile_pool(name="hbkp", bufs=thb, space="PSUM"))

    def panel_dram(d, g):
        # [t_sz, W] column slice viewed as [p, j, c] (j = t-block)
        return d[:, w_sz * g : w_sz * (g + 1)].rearrange("(j p) c -> p j c", p=P)

    for g in range(ng):
        fp = f_pool.tile([P, tb, w_sz], fp32, tag="fpanel")
        nc.sync.dma_start(fp[:], panel_dram(f_d, g))
        zp = z_pool.tile([P, tb, w_sz], fp32, tag="zpanel")
        nc.sync.dma_start(zp[:], panel_dram(z_d, g))
        bp = b_pool.tile([P, tb, w_sz], bf16, tag="bpanel")
        fh = fh_pool.tile([P, tb, w_sz], bf16, tag="fhpanel")
        fl = fl_pool.tile([P, tb, w_sz], bf16, tag="flpanel")
        hp = h_pool.tile([P, tb, w_sz], fp32, tag="hpanel")

        for j in range(tb):
            # bneg = (f - 1) * z, quantized to bf16
            nc.vector.scalar_tensor_tensor(
                bp[:, j], fp[:, j], 1.0, zp[:, j], op0=su, op1=mu
            )
        # f = f_hi + f_lo bf16 limb split on the (otherwise idle) GPSIMD
        nc.gpsimd.tensor_copy(fh[:], fp[:])
        nc.gpsimd.tensor_tensor(fl[:], fp[:], fh[:], op=su)

        prev_htr = [None] * nb
        for half in range(n_halves):
            hbks = []
            for jj in range(thb):
                hbk = hbk_p_pool.tile(
                    [P, w_sz], fp32, tag="hbkp", name=f"hbk_{g}_{half}_{jj}"
                )
                hbks.append(hbk)
            for i in range(nb):
                ftr_p = ftr_p_pool.tile([P, scan_len], fp32, tag="ftrp")
                btr_p = btr_p_pool.tile([P, scan_len], fp32, tag="btrp")
                for jj in range(thb):
                    j = half * thb + jj
                    sl = slice(P * jj, P * (jj + 1))
                    ni = slice(P * i, P * (i + 1))
                    # f.T = f_hi.T + f_lo.T accumulated in PSUM (regular
                    # matmuls against the bf16 identity: block.T @ I)
                    nc.tensor.matmul(
                        ftr_p[:, sl], fh[:, j, ni], identb[:], start=True, stop=False
                    )
                    nc.tensor.matmul(
                        ftr_p[:, sl], fl[:, j, ni], identb[:], start=False, stop=True
                    )
                    nc.tensor.matmul(
                        btr_p[:, sl], bp[:, j, ni], identb[:], start=True, stop=True
                    )
                btr_s = btr_s_pool.tile([P, scan_len], fp32, tag="btrs")
                nc.scalar.copy(btr_s[:], btr_p[:])
                htr = htr_pool.tile([P, scan_len], fp32, tag="htr")
                init = 0.0 if half == 0 else prev_htr[i][:, scan_len - 1 : scan_len]
                # state = (f * state) - bneg == f*state + (1-f)*z
                nc.vector.tensor_tensor_scan(
                    htr[:], ftr_p[:], btr_s[:], init, op0=mu, op1=su
                )
                prev_htr[i] = htr
                for jj in range(thb):
                    nc.tensor.transpose(
                        hbks[jj][:, P * i : P * (i + 1)],
                        htr[:, P * jj : P * (jj + 1)],
                        ident[:],
                    )
            for jj in range(thb):
                j = half * thb + jj
                nc.scalar.copy(hp[:, j], hbks[jj][:])

        nc.sync.dma_start(panel_dram(h_d, g), hp[:])


def build_program(t_sz=T, n_sz=N, w_sz=W_FULL):
    import concourse.tile as tile
    from concourse import bacc, mybir

    nc = bacc.Bacc(
        "TRN2",
        target_bir_lowering=False,
        debug=False,
        enable_asserts=False,
        num_devices=NCORES,
    )
    fp32 = mybir.dt.float32
    bf16 = mybir.dt.bfloat16
    f_d = nc.dram_tensor("f", [t_sz, n_sz], fp32, kind="ExternalInput").ap()
    z_d = nc.dram_tensor("z", [t_sz, n_sz], fp32, kind="ExternalInput").ap()
    i_d = nc.dram_tensor("ident", [P, P], fp32, kind="ExternalInput").ap()
    ib_d = nc.dram_tensor("identb", [P, P], bf16, kind="ExternalInput").ap()
    h_d = nc.dram_tensor("h", [t_sz, n_sz], fp32, kind="ExternalOutput").ap()
    with tile.TileContext(nc) as tc:
        with ExitStack() as ctx:
            build_forget_mult(tc, h_d, f_d, z_d, i_d, ib_d, ctx, t_sz, n_sz, w_sz)
    nc.compile()
    return nc


_compiled = None


def _get_program():
    global _compiled
    if _compiled is None:
        _compiled = build_program()
    return _compiled


def kernel(f, z, _trace=False):
    import ml_dtypes
    from concourse.bass_utils import run_bass_kernel_spmd

    f = np.asarray(f, dtype=np.float32)
    z = np.asarray(z, dtype=np.float32)
    assert f.shape == (T, B, H) and z.shape == (T, B, H)

    nc = _get_program()
    ident = np.eye(P, dtype=np.float32)
    identb = np.eye(P).astype(ml_dtypes.bfloat16)
    in_maps = []
    for c in range(NCORES):
        fc = np.ascontiguousarray(f[:, c * BPC : (c + 1) * BPC, :]).reshape(T, N)
        zc = np.ascontiguousarray(z[:, c * BPC : (c + 1) * BPC, :]).reshape(T, N)
        in_maps.append({"f": fc, "z": zc, "ident": ident, "identb": identb})

    kres = run_bass_kernel_spmd(nc, in_maps, list(range(NCORES)), trace=_trace)
    out = np.empty((T, B, H), dtype=np.float32)
    for c in range(NCORES):
        out[:, c * BPC : (c + 1) * BPC, :] = kres.results[c]["h"].reshape(T, BPC, H)
    if _trace:
        return out, kres
    return out


# revision 6
# speedup vs baseline: 1.9062x; 1.9062x over previous
"""ForgetMult (h_t = f_t*h_{t-1} + (1-f_t)*z_t) on 8 TRN2 NeuronCores.

Full inputs f, z: [T=1024, B=32, H=1024] f32. Output h: [T, B, H].

Sharding: batch dim across the 8 cores (4 batches/core), no communication.
Per core the problem is [T=1024, N=4096] with an independent linear
recurrence along T for each of the N columns.

Per-core dataflow (per n-group of W=512 columns):
  - one DMA per tensor brings the whole [T, W] panel in as a
    [128, T/128, W] t-block-interleaved SBUF tile (2 KiB rows)
  - DVE: bneg = (f - 1) * z -> bf16 (one scalar_tensor_tensor op)
  - PE transpose-mode 128x128 block transposes into PSUM. Transpose cost
    is per-instruction (~276 ns) and dtype-insensitive, so bf16 tensors
    are transposed as fp32-bitcast PAIRS of adjacent n columns — half the
    instructions. f stays fp32 (full precision for the recurrence
    coefficients); its blocks are split into even/odd n columns via
    stride-2 APs so partition labeling matches the packed pairs.
  - DVE: two tensor_tensor_scans per packed block (even/odd columns via
    stride-2 APs): state = f*state - bneg, fp32 state, bf16 stored h.
    data0 = f_tr straight from PSUM; data1 = bneg_tr copied PSUM->SBUF
    by ACT (scan operands cannot both live in PSUM).
  - scans write htr n-pair-interleaved; PE transposes htr as fp32 pairs
    back to [t, n] (again half the instructions), ACT copies PSUM->SBUF
    with bf16->fp32 cast, one DMA per group writes h out.

Precision: f and the scan state are fp32; bneg and stored h are bf16
(additive input and output quantization only, no compounding) ->
~1.6e-3 relative error on h.
"""

from contextlib import ExitStack

import numpy as np

T, B, H = 1024, 32, 1024
NCORES = 8
BPC = B // NCORES  # 4 batches per core
N = BPC * H  # 4096 recurrence columns per core
P = 128

W_FULL = 512  # panel width (columns per n-group)


def build_forget_mult(tc, h_d, f_d, z_d, i_d, ctx, t_sz, n_sz, w_sz):
    """Emit the per-core Tile program. f_d/z_d/h_d are DRAM APs [t_sz, n_sz]."""
    import concourse.bass as bass
    from concourse import mybir

    nc = tc.nc
    fp32 = mybir.dt.float32
    bf16 = mybir.dt.bfloat16
    su = mybir.AluOpType.subtract
    mu = mybir.AluOpType.mult

    tb = t_sz // P  # t-blocks (8)
    ng = n_sz // w_sz  # n-groups (8)
    npair = w_sz // (2 * P)  # packed pair-blocks per group (2)
    n_halves = 2  # scan chunks along T
    thb = tb // n_halves  # t-blocks per half (4)
    scan_len = thb * P  # 512
    assert t_sz % P == 0 and n_sz % w_sz == 0 and w_sz % (2 * P) == 0
    assert tb % n_halves == 0

    const_pool = ctx.enter_context(tc.tile_pool(name="const", bufs=1))
    ident = const_pool.tile([P, P], fp32)
    nc.sync.dma_start(ident[:], i_d[:])

    f_pool = ctx.enter_context(tc.tile_pool(name="fpanel", bufs=2))
    z_pool = ctx.enter_context(tc.tile_pool(name="zpanel", bufs=2))
    b_pool = ctx.enter_context(tc.tile_pool(name="bpanel", bufs=2))
    h_pool = ctx.enter_context(tc.tile_pool(name="hpanel", bufs=2))
    btr_s_pool = ctx.enter_context(tc.tile_pool(name="btrs", bufs=2))
    htr_pool = ctx.enter_context(tc.tile_pool(name="htr", bufs=3 * npair))
    ftre_pool = ctx.enter_context(tc.tile_pool(name="ftre", bufs=1, space="PSUM"))
    ftro_pool = ctx.enter_context(tc.tile_pool(name="ftro", bufs=1, space="PSUM"))
    btr_p_pool = ctx.enter_context(tc.tile_pool(name="btrp", bufs=2, space="PSUM"))
    hbk_p_pool = ctx.enter_context(tc.tile_pool(name="hbkp", bufs=thb, space="PSUM"))

    def panel_dram(d, g):
        # [t_sz, W] column slice viewed as [p, j, c] (j = t-block)
        return d[:, w_sz * g : w_sz * (g + 1)].rearrange("(j p) c -> p j c", p=P)

    for g in range(ng):
        fp = f_pool.tile([P, tb, w_sz], fp32, tag="fpanel")
        nc.sync.dma_start(fp[:], panel_dram(f_d, g))
        zp = z_pool.tile([P, tb, w_sz], fp32, tag="zpanel")
        nc.sync.dma_start(zp[:], panel_dram(z_d, g))
        bp = b_pool.tile([P, tb, w_sz], bf16, tag="bpanel")
        hp = h_pool.tile([P, tb, w_sz], fp32, tag="hpanel")

        for j in range(tb):
            # bneg = (f - 1) * z, quantized to bf16
            nc.vector.scalar_tensor_tensor(
                bp[:, j], fp[:, j], 1.0, zp[:, j], op0=su, op1=mu
            )

        prev_htr = [None] * npair
        for half in range(n_halves):
            hbks = []
            for jj in range(thb):
                hbk = hbk_p_pool.tile(
                    [P, w_sz // 2], fp32, tag="hbkp", name=f"hbk_{g}_{half}_{jj}"
                )
                hbks.append(hbk)
            for q in range(npair):
                cs = slice(2 * P * q, 2 * P * (q + 1))  # 256 columns of the group
                ftr_e = ftre_pool.tile([P, scan_len], fp32, tag="ftre")
                ftr_o = ftro_pool.tile([P, scan_len], fp32, tag="ftro")
                btr_p = btr_p_pool.tile([P, scan_len], fp32, tag="btrp")
                for jj in range(thb):
                    j = half * thb + jj
                    ts_ = slice(P * jj, P * (jj + 1))
                    nc.tensor.transpose(
                        ftr_e[:, ts_], fp[:, j, cs][:, 0::2], ident[:]
                    )
                    nc.tensor.transpose(
                        ftr_o[:, ts_], fp[:, j, cs][:, 1::2], ident[:]
                    )
                    # packed pair transpose: [128 t, 128 fp32 words]
                    nc.tensor.transpose(
                        btr_p[:, ts_], bp[:, j, cs].bitcast(fp32), ident[:]
                    )
                btr_s = btr_s_pool.tile([P, 2 * scan_len], bf16, tag="btrs")
                nc.scalar.copy(btr_s[:], btr_p[:].bitcast(bf16))
                htr = htr_pool.tile([P, 2 * scan_len], bf16, tag="htr")
                if half == 0:
                    init_e, init_o = 0.0, 0.0
                else:
                    pv = prev_htr[q]
                    init_e = pv[:, 2 * scan_len - 2 : 2 * scan_len - 1]
                    init_o = pv[:, 2 * scan_len - 1 : 2 * scan_len]
                # state = (f * state) - bneg == f*state + (1-f)*z
                nc.vector.tensor_tensor_scan(
                    htr[:, 0::2], ftr_e[:], btr_s[:, 0::2], init_e, op0=mu, op1=su
                )
                nc.vector.tensor_tensor_scan(
                    htr[:, 1::2], ftr_o[:], btr_s[:, 1::2], init_o, op0=mu, op1=su
                )
                prev_htr[q] = htr
                htr_w = htr[:].bitcast(fp32)  # [128, scan_len] packed pairs
                for jj in range(thb):
                    nc.tensor.transpose(
                        hbks[jj][:, P * q : P * (q + 1)],
                        htr_w[:, P * jj : P * (jj + 1)],
                        ident[:],
                    )
            for jj in range(thb):
                j = half * thb + jj
                nc.scalar.copy(hp[:, j], hbks[jj][:].bitcast(bf16))

        nc.sync.dma_start(panel_dram(h_d, g), hp[:])


def build_program(t_sz=T, n_sz=N, w_sz=W_FULL):
    import concourse.tile as tile
    from concourse import bacc, mybir

    nc = bacc.Bacc(
        "TRN2",
        target_bir_lowering=False,
        debug=False,
        enable_asserts=False,
        num_devices=NCORES,
    )
    fp32 = mybir.dt.float32
    f_d = nc.dram_tensor("f", [t_sz, n_sz], fp32, kind="ExternalInput").ap()
    z_d = nc.dram_tensor("z", [t_sz, n_sz], fp32, kind="ExternalInput").ap()
    i_d = nc.dram_tensor("ident", [P, P], fp32, kind="ExternalInput").ap()
    h_d = nc.dram_tensor("h", [t_sz, n_sz], fp32, kind="ExternalOutput").ap()
    with tile.TileContext(nc) as tc:
        with ExitStack() as ctx:
            build_forget_mult(tc, h_d, f_d, z_d, i_d, ctx, t_sz, n_sz, w_sz)
    nc.compile()
    return nc


_compiled = None


def _get_program():
    global _compiled
    if _compiled is None:
        _compiled = build_program()
    return _compiled


def kernel(f, z, _trace=False):
    from concourse.bass_utils import run_bass_kernel_spmd

    f = np.asarray(f, dtype=np.float32)
    z = np.asarray(z, dtype=np.float32)
    assert f.shape == (T, B, H) and z.shape == (T, B, H)

    nc = _get_program()
    ident = np.eye(P, dtype=np.float32)
    in_maps = []
    for c in range(NCORES):
        fc = np.ascontiguousarray(f[:, c * BPC : (c + 1) * BPC, :]).reshape(T, N)
        zc = np.ascontiguousarray(z[:, c * BPC : (c + 1) * BPC, :]).reshape(T, N)
        in_maps.append({"f": fc, "z": zc, "ident": ident})

    kres = run_bass_kernel_spmd(nc, in_maps, list(range(NCORES)), trace=_trace)
    out = np.empty((T, B, H), dtype=np.float32)
    for c in range(NCORES):
        out[:, c * BPC : (c + 1) * BPC, :] = kres.results[c]["h"].reshape(T, BPC, H)
    if _trace:
        return out, kres
    return out
